# revision 1
# baseline (speedup 1.0000x reference)
"""Trainium2 Bass kernel for nn_KineticEquation (gnn_message_passing).

Reference computation:
    contrib_1 = y[:, i1r] * rate1                 # [B, R1]
    contrib_2 = y[:, i2r0] * y[:, i2r1] * rate2   # [B, R2]
    y_out = scatter_add(contrib_1 -> i1p) + scatter_add(contrib_2 -> i2p)

Strategy (8 NeuronCores, full batch per core, reactions sharded by product
tile p//128 so each core owns one 128-row slice of y_out^T):

  * First-order term is a dense matmul: y_out1^T = W1[:, tile]^T @ y^T where
    W1[s, p] = sum(rate1 over reactions r=s, p=p). 8 PE matmuls per core.
  * Second-order reactions are processed in chunks of 128. All reactions in
    a chunk share (T0 = r0//128, T1 = r1//128). Per chunk:
       g0 = G0^T @ yT[T0]   (PE, one-hot lhsT with rate folded in; PSUM)
       g1 = G1^T @ yT[T1]   (PE, one-hot lhsT; PSUM)
       g0s = copy(g0)       (ACT, PSUM->SBUF)
       z   = g0s * g1       (DVE, -> SBUF fp32r)
       acc += SC^T @ z      (PE, one-hot scatter into the persistent PSUM
                             accumulator holding y_out^T[tile])
    The one-hot matrices are built on the host and streamed from HBM
    (they are only used once each; fp32r keeps ~11 mantissa bits which is
    ample since all contributions are positive).

All arithmetic on [B, R]-sized data happens on device; the host only does
O(R) index preprocessing and O(S^2) weight layout.
"""

import math

import numpy as np

import concourse.tile as tile
from concourse import bacc, mybir
from concourse.bass_utils import run_bass_kernel_spmd

F32 = mybir.dt.float32
F32R = mybir.dt.float32r

NCORES = 8
P = 128           # partitions / tile edge
S = 1024          # species
NT = S // P       # species tiles (8)
B = 512           # batch
GRP = 4           # chunks per DMA group


def _preprocess(y_in, i1r, i1p, r1, i2r0, i2r1, i2p, r2):
    """Host-side index preprocessing. Returns per-core input dicts + schedule."""
    i1r = np.asarray(i1r).astype(np.int64)
    i1p = np.asarray(i1p).astype(np.int64)
    i2r0 = np.asarray(i2r0).astype(np.int64)
    i2r1 = np.asarray(i2r1).astype(np.int64)
    i2p = np.asarray(i2p).astype(np.int64)
    r1 = np.asarray(r1).astype(np.float32)
    r2 = np.asarray(r2).astype(np.float32)

    # Dense first-order matrix W1[s, p] = sum of rates
    W1 = np.zeros((S, S), np.float32)
    np.add.at(W1, (i1r, i1p), r1)

    yT = np.ascontiguousarray(np.asarray(y_in, np.float32).T)  # [S, B]

    # Shard second-order reactions by product tile
    core_of = i2p >> 7
    T0 = i2r0 >> 7
    T1 = i2r1 >> 7
    binid = (T0 << 3) | T1  # 0..63

    counts = np.zeros((NCORES, NT * NT), np.int64)
    for c in range(NCORES):
        counts[c] = np.bincount(binid[core_of == c], minlength=NT * NT)
    maxc = counts.max(axis=0)
    nch_b = np.ceil(maxc / P).astype(np.int64)          # chunks per bin
    base_b = np.zeros(NT * NT + 1, np.int64)
    base_b[1:] = np.cumsum(nch_b)
    nchunk = int(base_b[-1])
    ngroup = math.ceil(nchunk / GRP)
    nchpad = ngroup * GRP

    sched = []  # (T0, T1) per chunk
    for b in range(NT * NT):
        sched.extend([(b >> 3, b & 7)] * int(nch_b[b]))

    in_maps = []
    for c in range(NCORES):
        sel = core_of == c
        bsel = binid[sel]
        order = np.argsort(bsel, kind="stable")
        bs = bsel[order]
        r0l = (i2r0[sel] & 127)[order]
        r1l = (i2r1[sel] & 127)[order]
        pl = (i2p[sel] & 127)[order]
        rr = r2[sel][order]
        bin_start = np.zeros(NT * NT, np.int64)
        cnt = np.bincount(bs, minlength=NT * NT)
        bin_start[1:] = np.cumsum(cnt)[:-1]
        pos = np.arange(len(bs)) - bin_start[bs]
        chunk = base_b[bs] + (pos >> 7)
        col = pos & 127

        G0 = np.zeros((nchpad, P, P), np.float32)
        G1 = np.zeros((nchpad, P, P), np.float32)
        SC = np.zeros((nchpad, P, P), np.float32)
        G0[chunk, r0l, col] = rr
        G1[chunk, r1l, col] = 1.0
        SC[chunk, col, pl] = 1.0

        def grp(x):
            return np.ascontiguousarray(
                x.reshape(ngroup, GRP, P, P).transpose(0, 2, 1, 3).reshape(ngroup, P, GRP * P)
            )

        in_maps.append(
            dict(
                yT=yT,
                W1g=np.ascontiguousarray(W1[:, c * P:(c + 1) * P]),  # [S, P]
                G0=grp(G0),
                G1=grp(G1),
                SCT=grp(SC),
            )
        )
    return in_maps, sched, nchunk, ngroup


def _build(nchunk, ngroup, sched, reps=1, bufs_oh=4, bufs_g0=3, bufs_g1=3, lag=2, skip=(), w1_last=False, warmup=0, npre=0):
    nc = bacc.Bacc("TRN2", target_bir_lowering=False, debug=False, num_devices=NCORES)

    yT_d = nc.dram_tensor("yT", [S, B], F32R, kind="ExternalInput").ap()
    w1_d = nc.dram_tensor("W1g", [S, P], F32R, kind="ExternalInput").ap()
    g0_d = nc.dram_tensor("G0", [ngroup, P, GRP * P], F32R, kind="ExternalInput").ap()
    g1_d = nc.dram_tensor("G1", [ngroup, P, GRP * P], F32R, kind="ExternalInput").ap()
    sc_d = nc.dram_tensor("SCT", [ngroup, P, GRP * P], F32R, kind="ExternalInput").ap()
    out_d = nc.dram_tensor("out", [P, B], F32, kind="ExternalOutput").ap()

    with tile.TileContext(nc) as tc:
        with (
            tc.tile_pool(name="res", bufs=1) as res,
            tc.tile_pool(name="oh", bufs=bufs_oh) as ohp,
            tc.tile_pool(name="work", bufs=3) as wp,
            tc.tile_pool(name="acc", bufs=1, space="PSUM") as accp,
            tc.tile_pool(name="gp0", bufs=bufs_g0, space="PSUM") as gp0p,
            tc.tile_pool(name="gp1", bufs=bufs_g1, space="PSUM") as gp1p,
        ):
            # Optional PE warmup: tiny dependency-free matmuls that run
            # during the initial DMA window so the HAM clock gate releases
            # before the first real matmul.
            if warmup:
                wt = res.tile([P, P], F32R, tag="warm")
                nc.vector.memset(wt[:], 0.0)
                wps = accp.tile([P, 8], F32, space="PSUM", tag="warmps")
                for _ in range(warmup):
                    nc.tensor.matmul(wps[:], lhsT=wt[:], rhs=wt[:, :8],
                                     start=True, stop=True)

            # Pre-issue the first `npre` one-hot groups ahead of the
            # resident loads so chunk 0's data is not queued behind 2.5MB
            # of y/W1 it does not need yet.
            pre = []
            for gi in range(min(npre, ngroup)):
                pg0 = ohp.tile([P, GRP * P], F32R, tag="g0g")
                pg1 = ohp.tile([P, GRP * P], F32R, tag="g1g")
                psc = ohp.tile([P, GRP * P], F32R, tag="scg")
                nc.sync.dma_start(pg0[:], g0_d[gi])
                nc.sync.dma_start(pg1[:], g1_d[gi])
                nc.sync.dma_start(psc[:], sc_d[gi])
                pre.append((pg0, pg1, psc))

            # Resident tiles: y^T species tiles and W1 slice
            yts = []
            for t in range(NT):
                yt = res.tile([P, B], F32R, tag=f"yt{t}")
                nc.sync.dma_start(yt[:], yT_d[t * P:(t + 1) * P, :])
                yts.append(yt)
            w1t = res.tile([P, NT * P], F32R, tag="w1")
            for t in range(NT):
                nc.sync.dma_start(w1t[:, t * P:(t + 1) * P], w1_d[t * P:(t + 1) * P, :])

            def one_pass():
                acc = accp.tile([P, B], F32, space="PSUM", tag="acc")
                first_acc = [True]

                def acc_mm(lhsT, rhs, stop=False):
                    nc.tensor.matmul(acc[:], lhsT=lhsT, rhs=rhs,
                                     start=first_acc[0], stop=stop)
                    first_acc[0] = False

                def w1_mms(last_stops=False):
                    # First-order: acc += sum_t W1g[tile t]^T @ yT[t]
                    for t in range(NT):
                        acc_mm(w1t[:, t * P:(t + 1) * P], yts[t][:],
                               stop=(last_stops and t == NT - 1))

                if not w1_last:
                    w1_mms()

                # Second-order chunks, software-pipelined by `lag` chunks so
                # the scatter matmul (which waits on DVE) never blocks gathers.
                from collections import deque
                pending = deque()
                for c in range(nchunk):
                    t0, t1 = sched[c]
                    gi, k = divmod(c, GRP)
                    if k == 0:
                        if pre and _rep_is_first[0] and gi < len(pre):
                            g0g, g1g, scg = pre[gi]
                        else:
                            g0g = ohp.tile([P, GRP * P], F32R, tag="g0g")
                            g1g = ohp.tile([P, GRP * P], F32R, tag="g1g")
                            scg = ohp.tile([P, GRP * P], F32R, tag="scg")
                            nc.sync.dma_start(g0g[:], g0_d[gi])
                            nc.sync.dma_start(g1g[:], g1_d[gi])
                            nc.sync.dma_start(scg[:], sc_d[gi])
                    cs = slice(k * P, (k + 1) * P)

                    g0p = gp0p.tile([P, B], F32, space="PSUM", tag="g0p")
                    g1p = gp1p.tile([P, B], F32, space="PSUM", tag="g1p")
                    nc.tensor.matmul(g0p[:], lhsT=g0g[:, cs], rhs=yts[t0][:],
                                     start=True, stop=True)
                    nc.tensor.matmul(g1p[:], lhsT=g1g[:, cs], rhs=yts[t1][:],
                                     start=True, stop=True)

                    g0s = wp.tile([P, B], F32, tag="g0s")
                    if "act" not in skip:
                        nc.scalar.mul(g0s[:], g0p[:], 1.0)
                    else:
                        nc.vector.tensor_copy(g0s[:], g0p[:])
                    z = wp.tile([P, B], F32R, tag="z")
                    if "dve" not in skip:
                        nc.vector.tensor_tensor(out=z[:], in0=g0s[:], in1=g1p[:],
                                                op=mybir.AluOpType.mult)
                    else:
                        nc.scalar.mul(z[:], g0s[:], 1.0)

                    if "scatter" in skip:
                        continue
                    pending.append((scg[:, cs], z))
                    if len(pending) > lag:
                        lh, zz = pending.popleft()
                        acc_mm(lh, zz[:])

                while pending:
                    lh, zz = pending.popleft()
                    acc_mm(lh, zz[:], stop=(not pending and not w1_last))

                if w1_last:
                    w1_mms(last_stops=True)

                outs = wp.tile([P, B], F32, tag="outs")
                nc.vector.tensor_copy(outs[:], acc[:])
                nc.sync.dma_start(out_d[:], outs[:])

            _rep_is_first = [True]
            for _rep in range(reps):
                one_pass()
                _rep_is_first[0] = False

    nc.compile()
    return nc


def _run(inputs, trace=False):
    in_maps, sched, nchunk, ngroup = _preprocess(
        inputs["y_in"], inputs["inds_1r"], inputs["inds_1p"], inputs["rate_1"],
        inputs["inds_2r0"], inputs["inds_2r1"], inputs["inds_2p"], inputs["rate_2"],
    )
    nc = _build(nchunk, ngroup, sched)
    res = None
    y_out = None
    last_exc = None
    for attempt in range(3):
        try:
            res = run_bass_kernel_spmd(nc, in_maps, list(range(NCORES)), trace=trace)
        except Exception as e:  # transient device wedges (NRT_EXEC_UNIT_...)
            last_exc = e
            import time as _time
            _time.sleep(2.0)
            continue
        y_out = np.empty((B, S), np.float32)
        for c in range(NCORES):
            y_out[:, c * P:(c + 1) * P] = res.results[c]["out"].T
        # guard against silent corruption from a wedged device
        if np.isfinite(y_out).all() and not (y_out == 0).all():
            break
        y_out = None
    if y_out is None:
        if last_exc is not None:
            raise last_exc
        raise RuntimeError("kernel produced non-finite/empty output on all attempts")
    return y_out, res


def kernel(**inputs) -> np.ndarray:
    return _run(inputs, trace=False)[0]

